# revision 4
# baseline (speedup 1.0000x reference)
"""Hardware-aware SNN (LIF, 2-layer, per-step weight noise) on 8 TRN2 cores.

Strategy
--------
Data-parallel over batch (4096 -> 8 x 512). Host reproduces the reference's
jax.random streams bit-exactly on CPU (threefry is backend-invariant),
precomputes per-step noisy weights Wn1[t] = W1 + e1[t] (f32) and splits them
into bf16 hi + bf16 lo so the PE runs full-rate bf16 matmuls with ~2^-18
effective weight precision. Spikes are 0/1 -> exact in bf16.

Device kernel (per core), per step t:
  layer1: psum[m] = sum_k  Wn1hi/lo[t,k,m].T @ spikesT[t,k]   (H-part x B-free)
  recurrence (shifted membrane mt = mem - 10*b so bias vanishes;
              reset term r_{t+1} == spk_t exactly):
     u = 0.9*mt (ACT);  u += psum (DVE);  mt = u - spk_prev (DVE)
     spk = (mt > theta) per-partition theta = 1 - 10*b   (GPSIMD, bf16 out)
  layer2: 4 col-strip-packed matmuls (tile_position) x hi/lo into one psum,
          strip-sum + same recurrence on [10,512]; un-shift mem2 for output.
Outputs are written [10, T*B] and transposed on host.
"""
import sys

sys.path.insert(0, "/opt/trn_rl_repo")

import numpy as np
import ml_dtypes

T = 25
B, D, H, O = 4096, 784, 500, 10
BETA = 0.9
THRESHOLD = 1.0
NOISE_STD = 0.02
NCORES = 8
BS = B // NCORES  # 512 batch per core
KD = 112          # D contraction chunk (784 = 7*112)
NKD = 7
MCH = [(0, 128), (128, 256), (256, 384), (384, 500)]  # H chunks
BF16 = ml_dtypes.bfloat16

_CACHE = {}


def _build():
    import concourse.bass as bass
    import concourse.bacc as bacc
    import concourse.mybir as mybir
    import concourse.tile as tile

    f32 = mybir.dt.float32
    bf16 = mybir.dt.bfloat16
    Alu = mybir.AluOpType

    nc = bacc.Bacc("TRN2", target_bir_lowering=False, debug=False)

    sp_d = nc.dram_tensor("sp", [T, NKD, KD, BS], bf16, kind="ExternalInput")
    w1h_d = nc.dram_tensor("w1h", [T, NKD, KD, H], bf16, kind="ExternalInput")
    w1l_d = nc.dram_tensor("w1l", [T, NKD, KD, H], bf16, kind="ExternalInput")
    w2h_d = nc.dram_tensor("w2h", [T, 128, 40], bf16, kind="ExternalInput")
    w2l_d = nc.dram_tensor("w2l", [T, 128, 40], bf16, kind="ExternalInput")
    th1_d = nc.dram_tensor("th1", [128, 4], f32, kind="ExternalInput")
    m1i_d = nc.dram_tensor("m1i", [128, 4], f32, kind="ExternalInput")
    cst2_d = nc.dram_tensor("cst2", [O, 3], f32, kind="ExternalInput")
    spk2_d = nc.dram_tensor("spk2o", [O, T * BS], f32, kind="ExternalOutput")
    mem2_d = nc.dram_tensor("mem2o", [O, T * BS], f32, kind="ExternalOutput")

    with tile.TileContext(nc) as tc:
        with (
            tc.tile_pool(name="wpool", bufs=4) as wpool,
            tc.tile_pool(name="state", bufs=1) as state,
            tc.tile_pool(name="upool", bufs=2) as upool,
            tc.tile_pool(name="s1pool", bufs=2) as s1pool,
            tc.tile_pool(name="s2pool", bufs=2) as s2pool,
            tc.tile_pool(name="ps1", bufs=6, space="PSUM") as ps1,
            tc.tile_pool(name="ps2", bufs=2, space="PSUM") as ps2,
        ):
            # ---- constants / state init ----
            th1 = state.tile([128, 4], f32, tag="th1")
            m1i = state.tile([128, 4], f32, tag="m1i")
            cst2 = state.tile([O, 3], f32, tag="cst2")  # cols: th2, m2i, 10*b2
            w2h = state.tile([128, 40], bf16, tag="w2h_all")
            w2l = state.tile([128, 40], bf16, tag="w2l_all")
            nc.sync.dma_start(th1[:], th1_d.ap())
            nc.sync.dma_start(m1i[:], m1i_d.ap())
            nc.sync.dma_start(cst2[:], cst2_d.ap())

            m1 = []
            s1_prev = []
            for m, (h0, h1) in enumerate(MCH):
                mw = h1 - h0
                mt = state.tile([128, BS], f32, tag=f"m1_{m}")
                nc.vector.memset(mt[:mw], 0.0)
                nc.vector.tensor_scalar(
                    mt[:mw], mt[:mw], m1i[:mw, m : m + 1], None, Alu.add
                )
                m1.append(mt)
                sp0 = s1pool.tile([128, BS], bf16, tag=f"s1_{m}")
                nc.gpsimd.memset(sp0[:mw], 0.0)
                s1_prev.append(sp0)

            m2 = state.tile([O, BS], f32, tag="m2")
            nc.vector.memset(m2[:], 0.0)
            nc.vector.tensor_scalar(m2[:], m2[:], cst2[:, 1:2], None, Alu.add)
            s2_prev = s2pool.tile([O, BS], f32, tag="s2")
            nc.vector.memset(s2_prev[:], 0.0)

            for t in range(T):
                # ---- load this step's operands ----
                wh = wpool.tile([KD, NKD * H], bf16, tag="w1h")
                wl = wpool.tile([KD, NKD * H], bf16, tag="w1l")
                sp = wpool.tile([KD, NKD * BS], bf16, tag="sp")
                for k in range(NKD):
                    nc.sync.dma_start(wh[:, k * H : (k + 1) * H], w1h_d.ap()[t, k])
                    nc.sync.dma_start(wl[:, k * H : (k + 1) * H], w1l_d.ap()[t, k])
                    nc.sync.dma_start(sp[:, k * BS : (k + 1) * BS], sp_d.ap()[t, k])
                w2ht = wpool.tile([128, 40], bf16, tag="w2h")
                w2lt = wpool.tile([128, 40], bf16, tag="w2l")
                nc.sync.dma_start(w2ht[:], w2h_d.ap()[t])
                nc.sync.dma_start(w2lt[:], w2l_d.ap()[t])

                s1_new = []
                for m, (h0, h1) in enumerate(MCH):
                    mw = h1 - h0
                    ps = ps1.tile([128, BS], f32, tag="ps1")
                    for k in range(NKD):
                        for wsrc, first in ((wh, k == 0), (wl, False)):
                            nc.tensor.matmul(
                                ps[:mw],
                                wsrc[:, k * H + h0 : k * H + h1],
                                sp[:, k * BS : (k + 1) * BS],
                                start=first,
                                stop=(k == NKD - 1 and wsrc is wl),
                            )
                    # recurrence: m1' = 0.9*m1 + cur - spk_prev ; spk = m1' > th
                    u = upool.tile([128, BS], f32, tag=f"u_{m}")
                    nc.scalar.mul(u[:mw], m1[m][:mw], BETA)
                    nc.vector.tensor_tensor(u[:mw], u[:mw], ps[:mw], op=Alu.add)
                    nc.vector.tensor_tensor(
                        m1[m][:mw], u[:mw], s1_prev[m][:mw], op=Alu.subtract
                    )
                    ns = s1pool.tile([128, BS], bf16, tag=f"s1_{m}")
                    nc.gpsimd.tensor_scalar(
                        ns[:mw], m1[m][:mw], th1[:mw, m : m + 1], None, Alu.is_gt
                    )
                    s1_new.append(ns)

                # ---- layer 2: col-strip packed matmuls ----
                p2 = ps2.tile([128, BS], f32, tag="ps2")
                for j in range(4):
                    kj = MCH[j][1] - MCH[j][0]
                    for wsrc, first in ((w2ht, True), (w2lt, False)):
                        nc.tensor.matmul(
                            p2[32 * j : 32 * j + O, :],
                            wsrc[:kj, j * 10 : (j + 1) * 10],
                            s1_new[j][:kj, :],
                            start=first,
                            stop=not first,
                            tile_position=(0, 32 * j),
                        )
                a = upool.tile([O, BS], f32, tag="l2a")
                nc.scalar.copy(a[:], p2[0:O, :])
                nc.vector.tensor_tensor(a[:], a[:], p2[32 : 32 + O, :], op=Alu.add)
                nc.vector.tensor_tensor(a[:], a[:], p2[64 : 64 + O, :], op=Alu.add)
                nc.vector.tensor_tensor(a[:], a[:], p2[96 : 96 + O, :], op=Alu.add)
                u2 = upool.tile([O, BS], f32, tag="u2")
                nc.scalar.mul(u2[:], m2[:], BETA)
                nc.vector.tensor_tensor(u2[:], u2[:], a[:], op=Alu.add)
                nc.vector.tensor_tensor(m2[:], u2[:], s2_prev[:], op=Alu.subtract)
                s2 = s2pool.tile([O, BS], f32, tag="s2")
                nc.vector.tensor_scalar(
                    s2[:], m2[:], cst2[:, 0:1], None, Alu.is_gt
                )
                mo = s2pool.tile([O, BS], f32, tag="m2o")
                nc.vector.tensor_scalar(
                    mo[:], m2[:], cst2[:, 2:3], None, Alu.add
                )
                nc.sync.dma_start(spk2_d.ap()[:, t * BS : (t + 1) * BS], s2[:])
                nc.sync.dma_start(mem2_d.ap()[:, t * BS : (t + 1) * BS], mo[:])
                s1_prev = s1_new
                s2_prev = s2

    nc.compile()
    return nc


_RNG_SCRIPT = r"""
import numpy as np, sys
import jax, jax.numpy as jnp
T,B,D,H,O = 25,4096,784,500,10
x = np.load(sys.argv[1])["x"]
key = jax.random.key(42)
k_sp, k_n1, k_n2 = jax.random.split(key, 3)
u = jax.random.uniform(k_sp, (T,B,D), dtype=jnp.float32)
spikes = np.asarray(u) < x[None]
n1 = np.asarray(jax.random.normal(k_n1, (T,D,H), dtype=jnp.float32) * 0.02)
n2 = np.asarray(jax.random.normal(k_n2, (T,H,O), dtype=jnp.float32) * 0.02)
np.savez(sys.argv[2], spikes=spikes, n1=n1, n2=n2)
"""


def _host_rng(x):
    """jax threefry must run in a pure-CPU process: under the axon plugin the
    RNG ops get routed to the neuron backend and yield a different stream."""
    import subprocess, tempfile, os

    with tempfile.TemporaryDirectory() as td:
        xp, op = os.path.join(td, "x.npz"), os.path.join(td, "rng.npz")
        np.savez(xp, x=x)
        env = dict(os.environ, JAX_PLATFORMS="cpu")
        subprocess.run(
            [sys.executable, "-c", _RNG_SCRIPT, xp, op],
            env=env, check=True, capture_output=True,
        )
        d = np.load(op)
        return d["spikes"], d["n1"], d["n2"]


def _prep_inputs(x, W1, b1, W2, b2):
    """Reproduce reference RNG on CPU and build per-core device inputs."""
    spikes, n1, n2 = _host_rng(np.asarray(x, np.float32))

    Wn1 = W1[None] + n1  # [T,D,H] f32
    Wn2 = W2[None] + n2  # [T,H,O] f32
    w1h = Wn1.astype(BF16)
    w1l = (Wn1 - w1h.astype(np.float32)).astype(BF16)
    w1h = np.ascontiguousarray(w1h.reshape(T, NKD, KD, H))
    w1l = np.ascontiguousarray(w1l.reshape(T, NKD, KD, H))

    w2h_f = Wn2.astype(BF16)
    w2l_f = (Wn2 - w2h_f.astype(np.float32)).astype(BF16)
    w2h = np.zeros((T, 128, 40), BF16)
    w2l = np.zeros((T, 128, 40), BF16)
    for j, (h0, h1) in enumerate(MCH):
        kj = h1 - h0
        w2h[:, :kj, j * 10 : (j + 1) * 10] = w2h_f[:, h0:h1, :]
        w2l[:, :kj, j * 10 : (j + 1) * 10] = w2l_f[:, h0:h1, :]

    th1 = np.zeros((128, 4), np.float32)
    m1i = np.zeros((128, 4), np.float32)
    for m, (h0, h1) in enumerate(MCH):
        mw = h1 - h0
        th1[:mw, m] = THRESHOLD - 10.0 * b1[h0:h1]
        m1i[:mw, m] = -10.0 * b1[h0:h1]
    cst2 = np.stack(
        [THRESHOLD - 10.0 * b2, -10.0 * b2, 10.0 * b2], axis=1
    ).astype(np.float32)

    shared = {
        "w1h": w1h, "w1l": w1l, "w2h": w2h, "w2l": w2l,
        "th1": th1, "m1i": m1i, "cst2": cst2,
    }
    in_maps = []
    for c in range(NCORES):
        sl = spikes[:, c * BS : (c + 1) * BS, :]  # [T,BS,D]
        spT = np.ascontiguousarray(sl.transpose(0, 2, 1)).astype(BF16)
        in_maps.append({"sp": spT.reshape(T, NKD, KD, BS), **shared})
    return in_maps


def _install_trace_shim():
    """This image's antenv lacks axon_hooks; recreate the NTFF hook from
    trn_boot's ctypes wrapper so run_bass_kernel_spmd(trace=True) works."""
    import types
    from concourse import bass_utils

    if "antenv.axon_hooks" not in sys.modules:
        from trn_agent_boot.trn_boot import _ntff_profile_via_ctypes

        hook = _ntff_profile_via_ctypes("/opt/axon/libaxon_pjrt.so")
        m = types.ModuleType("antenv.axon_hooks")
        m.get_axon_ntff_profile_hook = lambda: hook
        m.set_axon_ntff_profile_hook = lambda h: None
        sys.modules["antenv.axon_hooks"] = m
    bass_utils.upload_artifacts = lambda tmpdir: tmpdir


def _run(inputs, trace=False, trace_kwargs=None):
    from concourse import bass_utils

    if trace:
        _install_trace_shim()
    if "nc" not in _CACHE:
        _CACHE["nc"] = _build()
    nc = _CACHE["nc"]
    in_maps = _prep_inputs(
        inputs["x"], inputs["W1"], inputs["b1"], inputs["W2"], inputs["b2"]
    )
    res = bass_utils.run_bass_kernel_spmd(
        nc,
        in_maps,
        core_ids=list(range(NCORES)),
        trace=trace,
        **(trace_kwargs or {}),
    )
    spk2 = np.empty((T, B, O), np.float32)
    mem2 = np.empty((T, B, O), np.float32)
    for c in range(NCORES):
        s = res.results[c]["spk2o"].reshape(O, T, BS).transpose(1, 2, 0)
        m = res.results[c]["mem2o"].reshape(O, T, BS).transpose(1, 2, 0)
        spk2[:, c * BS : (c + 1) * BS, :] = s
        mem2[:, c * BS : (c + 1) * BS, :] = m
    return (spk2, mem2), res


def kernel(x, W1, b1, W2, b2):
    (spk2, mem2), _ = _run(
        {"x": x, "W1": W1, "b1": b1, "W2": W2, "b2": b2}, trace=False
    )
    return spk2, mem2


# revision 10
# speedup vs baseline: 2.3170x; 2.3170x over previous
"""Hardware-aware SNN (LIF, 2-layer, per-step weight noise) on 8 TRN2 cores.

Strategy
--------
Data-parallel over batch (4096 -> 8 x 512). Host reproduces the reference's
jax.random streams bit-exactly on CPU (threefry is backend-invariant),
precomputes per-step noisy weights Wn1[t] = W1 + e1[t] (f32) and splits them
into bf16 hi + bf16 lo so the PE runs full-rate bf16 matmuls with ~2^-18
effective weight precision. Spikes are 0/1 -> exact in bf16.

Device kernel (per core), per step t:
  layer1: psum[m] = sum_k  Wn1hi/lo[t,k,m].T @ spikesT[t,k]   (H-part x B-free)
  recurrence (shifted membrane mt = mem - 10*b so bias vanishes;
              reset term r_{t+1} == spk_t exactly):
     u = 0.9*mt (ACT);  u += psum (DVE);  mt = u - spk_prev (DVE)
     spk = (mt > theta) per-partition theta = 1 - 10*b   (GPSIMD, bf16 out)
  layer2: 4 col-strip-packed matmuls (tile_position) x hi/lo into one psum,
          strip-sum + same recurrence on [10,512]; un-shift mem2 for output.
Outputs are written [10, T*B] and transposed on host.
"""
import sys

sys.path.insert(0, "/opt/trn_rl_repo")

import numpy as np
import ml_dtypes

T = 25
B, D, H, O = 4096, 784, 500, 10
BETA = 0.9
THRESHOLD = 1.0
NOISE_STD = 0.02
NCORES = 8
BS = B // NCORES  # 512 batch per core
KD = 112          # D contraction chunk (784 = 7*112)
NKD = 7
MCH = [(0, 128), (128, 256), (256, 384), (384, 500)]  # H chunks
BF16 = ml_dtypes.bfloat16

_CACHE = {}


def _build():
    import concourse.bass as bass
    import concourse.bacc as bacc
    import concourse.mybir as mybir
    import concourse.tile as tile

    f32 = mybir.dt.float32
    bf16 = mybir.dt.bfloat16
    Alu = mybir.AluOpType

    nc = bacc.Bacc("TRN2", target_bir_lowering=False, debug=False)

    sp_d = nc.dram_tensor("sp", [T, NKD, KD, BS], bf16, kind="ExternalInput")
    w1h_d = nc.dram_tensor("w1h", [T, NKD, KD, H], bf16, kind="ExternalInput")
    w1l_d = nc.dram_tensor("w1l", [T, NKD, KD, H], bf16, kind="ExternalInput")
    w2h_d = nc.dram_tensor("w2h", [T, 128, 40], bf16, kind="ExternalInput")
    w2l_d = nc.dram_tensor("w2l", [T, 128, 40], bf16, kind="ExternalInput")
    th1_d = nc.dram_tensor("th1", [128, 4], f32, kind="ExternalInput")
    m1i_d = nc.dram_tensor("m1i", [128, 4], f32, kind="ExternalInput")
    cst2_d = nc.dram_tensor("cst2", [O, 3], f32, kind="ExternalInput")
    nid_d = nc.dram_tensor("nid", [128, 128], bf16, kind="ExternalInput")
    spk2_d = nc.dram_tensor("spk2o", [O, T * BS], bf16, kind="ExternalOutput")
    mem2_d = nc.dram_tensor("mem2o", [O, T * BS], f32, kind="ExternalOutput")

    with tile.TileContext(nc) as tc:
        with (
            tc.tile_pool(name="wpool", bufs=4) as wpool,
            tc.tile_pool(name="state", bufs=1) as state,
            tc.tile_pool(name="upool", bufs=2) as upool,
            tc.tile_pool(name="s1pool", bufs=2) as s1pool,
            tc.tile_pool(name="s2pool", bufs=2) as s2pool,
            tc.tile_pool(name="ps1", bufs=6, space="PSUM") as ps1,
            tc.tile_pool(name="ps2", bufs=2, space="PSUM") as ps2,
        ):
            # ---- constants / state init ----
            th1 = state.tile([128, 4], f32, tag="th1")
            m1i = state.tile([128, 4], f32, tag="m1i")
            cst2 = state.tile([O, 3], f32, tag="cst2")  # cols: th2, m2i, 10*b2
            nid = state.tile([128, 128], bf16, tag="nid")
            nc.sync.dma_start(th1[:], th1_d.ap())
            nc.sync.dma_start(m1i[:], m1i_d.ap())
            nc.sync.dma_start(cst2[:], cst2_d.ap())
            nc.sync.dma_start(nid[:], nid_d.ap())

            m1 = []
            s1_prev = []
            for m, (h0, h1) in enumerate(MCH):
                mw = h1 - h0
                mt = state.tile([128, BS], f32, tag=f"m1_{m}")
                nc.vector.memset(mt[:mw], 0.0)
                nc.vector.tensor_scalar(
                    mt[:mw], mt[:mw], m1i[:mw, m : m + 1], None, Alu.add
                )
                m1.append(mt)
                sp0 = s1pool.tile([128, BS], bf16, tag=f"s1_{m}")
                nc.gpsimd.memset(sp0[:mw], 0.0)
                s1_prev.append(sp0)

            m2 = state.tile([O, BS], f32, tag="m2")
            nc.vector.memset(m2[:], 0.0)
            nc.vector.tensor_scalar(m2[:], m2[:], cst2[:, 1:2], None, Alu.add)
            s2_prev = s2pool.tile([O, BS], bf16, tag="s2")
            nc.vector.memset(s2_prev[:], 0.0)

            for t in range(T):
                # ---- load this step's operands ----
                wh = wpool.tile([KD, NKD * H], bf16, tag="w1h")
                wl = wpool.tile([KD, NKD * H], bf16, tag="w1l")
                sp = wpool.tile([KD, NKD * BS], bf16, tag="sp")
                for k in range(NKD):
                    nc.sync.dma_start(wh[:, k * H : (k + 1) * H], w1h_d.ap()[t, k])
                    nc.sync.dma_start(wl[:, k * H : (k + 1) * H], w1l_d.ap()[t, k])
                    nc.sync.dma_start(sp[:, k * BS : (k + 1) * BS], sp_d.ap()[t, k])
                w2ht = wpool.tile([128, 40], bf16, tag="w2h")
                w2lt = wpool.tile([128, 40], bf16, tag="w2l")
                nc.sync.dma_start(w2ht[:], w2h_d.ap()[t])
                nc.sync.dma_start(w2lt[:], w2l_d.ap()[t])

                s1_new = []
                for m, (h0, h1) in enumerate(MCH):
                    mw = h1 - h0
                    ps = ps1.tile([128, BS], f32, tag="ps1")
                    # reset term folded into PE: psum -= spk_prev (via -I)
                    nc.tensor.matmul(
                        ps[:mw], nid[:mw, :mw], s1_prev[m][:mw, :],
                        start=True, stop=False,
                    )
                    for k in range(NKD):
                        for wsrc, last in ((wh, False), (wl, k == NKD - 1)):
                            nc.tensor.matmul(
                                ps[:mw],
                                wsrc[:, k * H + h0 : k * H + h1],
                                sp[:, k * BS : (k + 1) * BS],
                                start=False,
                                stop=last,
                            )
                    # m1' = 0.9*m1 + (cur - spk_prev) ; spk = m1' > th
                    u = upool.tile([128, BS], f32, tag=f"u_{m}")
                    nc.scalar.mul(u[:mw], m1[m][:mw], BETA)
                    nc.vector.tensor_tensor(m1[m][:mw], u[:mw], ps[:mw], op=Alu.add)
                    ns = s1pool.tile([128, BS], bf16, tag=f"s1_{m}")
                    nc.vector.tensor_scalar(
                        ns[:mw], m1[m][:mw], th1[:mw, m : m + 1], None, Alu.is_gt
                    )
                    s1_new.append(ns)

                # ---- layer 2: sequential accumulation, M=10, N=512 ----
                p2 = ps2.tile([O, BS], f32, tag="ps2")
                nc.tensor.matmul(
                    p2[:], nid[:O, :O], s2_prev[:], start=True, stop=False
                )
                for j in range(4):
                    kj = MCH[j][1] - MCH[j][0]
                    for wsrc, last in ((w2ht, False), (w2lt, j == 3)):
                        nc.tensor.matmul(
                            p2[:],
                            wsrc[:kj, j * 10 : (j + 1) * 10],
                            s1_new[j][:kj, :],
                            start=False,
                            stop=last,
                        )
                u2 = upool.tile([O, BS], f32, tag="u2")
                nc.scalar.mul(u2[:], m2[:], BETA)
                nc.vector.tensor_tensor(m2[:], u2[:], p2[:], op=Alu.add)
                s2 = s2pool.tile([O, BS], bf16, tag="s2")
                nc.vector.tensor_scalar(
                    s2[:], m2[:], cst2[:, 0:1], None, Alu.is_gt
                )
                mo = s2pool.tile([O, BS], f32, tag="m2o")
                nc.vector.tensor_scalar(
                    mo[:], m2[:], cst2[:, 2:3], None, Alu.add
                )
                nc.sync.dma_start(spk2_d.ap()[:, t * BS : (t + 1) * BS], s2[:])
                nc.sync.dma_start(mem2_d.ap()[:, t * BS : (t + 1) * BS], mo[:])
                s1_prev = s1_new
                s2_prev = s2

    nc.compile()
    return nc


_RNG_SCRIPT = r"""
import numpy as np, sys
import jax, jax.numpy as jnp
T,B,D,H,O = 25,4096,784,500,10
x = np.load(sys.argv[1])["x"]
key = jax.random.key(42)
k_sp, k_n1, k_n2 = jax.random.split(key, 3)
u = jax.random.uniform(k_sp, (T,B,D), dtype=jnp.float32)
spikes = np.asarray(u) < x[None]
n1 = np.asarray(jax.random.normal(k_n1, (T,D,H), dtype=jnp.float32) * 0.02)
n2 = np.asarray(jax.random.normal(k_n2, (T,H,O), dtype=jnp.float32) * 0.02)
np.savez(sys.argv[2], spikes=spikes, n1=n1, n2=n2)
"""


def _host_rng(x):
    """jax threefry must run in a pure-CPU process: under the axon plugin the
    RNG ops get routed to the neuron backend and yield a different stream."""
    import subprocess, tempfile, os

    with tempfile.TemporaryDirectory() as td:
        xp, op = os.path.join(td, "x.npz"), os.path.join(td, "rng.npz")
        np.savez(xp, x=x)
        env = dict(os.environ, JAX_PLATFORMS="cpu")
        subprocess.run(
            [sys.executable, "-c", _RNG_SCRIPT, xp, op],
            env=env, check=True, capture_output=True,
        )
        d = np.load(op)
        return d["spikes"], d["n1"], d["n2"]


def _prep_inputs(x, W1, b1, W2, b2):
    """Reproduce reference RNG on CPU and build per-core device inputs."""
    spikes, n1, n2 = _host_rng(np.asarray(x, np.float32))

    Wn1 = W1[None] + n1  # [T,D,H] f32
    Wn2 = W2[None] + n2  # [T,H,O] f32
    w1h = Wn1.astype(BF16)
    w1l = (Wn1 - w1h.astype(np.float32)).astype(BF16)
    w1h = np.ascontiguousarray(w1h.reshape(T, NKD, KD, H))
    w1l = np.ascontiguousarray(w1l.reshape(T, NKD, KD, H))

    w2h_f = Wn2.astype(BF16)
    w2l_f = (Wn2 - w2h_f.astype(np.float32)).astype(BF16)
    w2h = np.zeros((T, 128, 40), BF16)
    w2l = np.zeros((T, 128, 40), BF16)
    for j, (h0, h1) in enumerate(MCH):
        kj = h1 - h0
        w2h[:, :kj, j * 10 : (j + 1) * 10] = w2h_f[:, h0:h1, :]
        w2l[:, :kj, j * 10 : (j + 1) * 10] = w2l_f[:, h0:h1, :]

    th1 = np.zeros((128, 4), np.float32)
    m1i = np.zeros((128, 4), np.float32)
    for m, (h0, h1) in enumerate(MCH):
        mw = h1 - h0
        th1[:mw, m] = THRESHOLD - 10.0 * b1[h0:h1]
        m1i[:mw, m] = -10.0 * b1[h0:h1]
    cst2 = np.stack(
        [THRESHOLD - 10.0 * b2, -10.0 * b2, 10.0 * b2], axis=1
    ).astype(np.float32)

    nid = np.zeros((128, 128), BF16)
    np.fill_diagonal(nid, BF16(-1.0))
    shared = {
        "w1h": w1h, "w1l": w1l, "w2h": w2h, "w2l": w2l,
        "th1": th1, "m1i": m1i, "cst2": cst2, "nid": nid,
    }
    in_maps = []
    for c in range(NCORES):
        sl = spikes[:, c * BS : (c + 1) * BS, :]  # [T,BS,D]
        spT = np.ascontiguousarray(sl.transpose(0, 2, 1)).astype(BF16)
        in_maps.append({"sp": spT.reshape(T, NKD, KD, BS), **shared})
    return in_maps


def _install_trace_shim():
    """This image's antenv lacks axon_hooks; recreate the NTFF hook from
    trn_boot's ctypes wrapper so run_bass_kernel_spmd(trace=True) works."""
    import types
    from concourse import bass_utils

    if "antenv.axon_hooks" not in sys.modules:
        from trn_agent_boot.trn_boot import _ntff_profile_via_ctypes

        hook = _ntff_profile_via_ctypes("/opt/axon/libaxon_pjrt.so")
        m = types.ModuleType("antenv.axon_hooks")
        m.get_axon_ntff_profile_hook = lambda: hook
        m.set_axon_ntff_profile_hook = lambda h: None
        sys.modules["antenv.axon_hooks"] = m
    bass_utils.upload_artifacts = lambda tmpdir: tmpdir


def _run(inputs, trace=False, trace_kwargs=None):
    from concourse import bass_utils

    if trace:
        _install_trace_shim()
    if "nc" not in _CACHE:
        _CACHE["nc"] = _build()
    nc = _CACHE["nc"]
    in_maps = _prep_inputs(
        inputs["x"], inputs["W1"], inputs["b1"], inputs["W2"], inputs["b2"]
    )
    res = bass_utils.run_bass_kernel_spmd(
        nc,
        in_maps,
        core_ids=list(range(NCORES)),
        trace=trace,
        **(trace_kwargs or {}),
    )
    spk2 = np.empty((T, B, O), np.float32)
    mem2 = np.empty((T, B, O), np.float32)
    for c in range(NCORES):
        s = res.results[c]["spk2o"].astype(np.float32).reshape(O, T, BS).transpose(1, 2, 0)
        m = res.results[c]["mem2o"].reshape(O, T, BS).transpose(1, 2, 0)
        spk2[:, c * BS : (c + 1) * BS, :] = s
        mem2[:, c * BS : (c + 1) * BS, :] = m
    return (spk2, mem2), res


def kernel(x, W1, b1, W2, b2):
    (spk2, mem2), _ = _run(
        {"x": x, "W1": W1, "b1": b1, "W2": W2, "b2": b2}, trace=False
    )
    return spk2, mem2


# revision 15
# speedup vs baseline: 2.4975x; 1.0779x over previous
"""Hardware-aware SNN (LIF, 2-layer, per-step weight noise) on 8 TRN2 cores.

Strategy
--------
Data-parallel over batch (4096 -> 8 x 512). Host reproduces the reference's
jax.random streams bit-exactly on CPU (threefry is backend-invariant),
precomputes per-step noisy weights Wn1[t] = W1 + e1[t] (f32) and splits them
into bf16 hi + bf16 lo so the PE runs full-rate bf16 matmuls with ~2^-18
effective weight precision. Spikes are 0/1 -> exact in bf16.

Device kernel (per core), per step t:
  layer1: psum[m] = sum_k  Wn1hi/lo[t,k,m].T @ spikesT[t,k]   (H-part x B-free)
  recurrence (shifted membrane mt = mem - 10*b so bias vanishes;
              reset term r_{t+1} == spk_t exactly):
     u = 0.9*mt (ACT);  u += psum (DVE);  mt = u - spk_prev (DVE)
     spk = (mt > theta) per-partition theta = 1 - 10*b   (GPSIMD, bf16 out)
  layer2: 4 col-strip-packed matmuls (tile_position) x hi/lo into one psum,
          strip-sum + same recurrence on [10,512]; un-shift mem2 for output.
Outputs are written [10, T*B] and transposed on host.
"""
import sys

sys.path.insert(0, "/opt/trn_rl_repo")

import numpy as np
import ml_dtypes

T = 25
B, D, H, O = 4096, 784, 500, 10
BETA = 0.9
THRESHOLD = 1.0
NOISE_STD = 0.02
NCORES = 8
BS = B // NCORES  # 512 batch per core
KD = 112          # D contraction chunk (784 = 7*112)
NKD = 7
MCH = [(0, 128), (128, 256), (256, 384), (384, 500)]  # H chunks
BF16 = ml_dtypes.bfloat16

_CACHE = {}


def _build():
    import concourse.bass as bass
    import concourse.bacc as bacc
    import concourse.mybir as mybir
    import concourse.tile as tile

    f32 = mybir.dt.float32
    bf16 = mybir.dt.bfloat16
    Alu = mybir.AluOpType

    nc = bacc.Bacc("TRN2", target_bir_lowering=False, debug=False)

    sp_d = nc.dram_tensor("sp", [T, KD, NKD * BS], bf16, kind="ExternalInput")
    w1h_d = nc.dram_tensor("w1h", [T, KD, NKD * H], bf16, kind="ExternalInput")
    w1l_d = nc.dram_tensor("w1l", [T, KD, NKD * H], bf16, kind="ExternalInput")
    w2h_d = nc.dram_tensor("w2h", [T, 128, 40], bf16, kind="ExternalInput")
    w2l_d = nc.dram_tensor("w2l", [T, 128, 40], bf16, kind="ExternalInput")
    th1_d = nc.dram_tensor("th1", [128, 4], f32, kind="ExternalInput")
    m1i_d = nc.dram_tensor("m1i", [128, 4], f32, kind="ExternalInput")
    cst2_d = nc.dram_tensor("cst2", [O, 3], f32, kind="ExternalInput")
    nid_d = nc.dram_tensor("nid", [128, 128], bf16, kind="ExternalInput")
    spk2_d = nc.dram_tensor("spk2o", [O, T * BS], bf16, kind="ExternalOutput")
    mem2_d = nc.dram_tensor("mem2o", [O, T * BS], f32, kind="ExternalOutput")

    with tile.TileContext(nc) as tc:
        with (
            tc.tile_pool(name="wpool", bufs=4) as wpool,
            tc.tile_pool(name="state", bufs=1) as state,
            tc.tile_pool(name="upool", bufs=2) as upool,
            tc.tile_pool(name="s1pool", bufs=2) as s1pool,
            tc.tile_pool(name="s2pool", bufs=2) as s2pool,
            tc.tile_pool(name="ps1", bufs=6, space="PSUM") as ps1,
            tc.tile_pool(name="ps2", bufs=2, space="PSUM") as ps2,
        ):
            # ---- constants / state init ----
            th1 = state.tile([128, 4], f32, tag="th1")
            m1i = state.tile([128, 4], f32, tag="m1i")
            cst2 = state.tile([O, 3], f32, tag="cst2")  # cols: th2, m2i, 10*b2
            nid = state.tile([128, 128], bf16, tag="nid")
            nc.sync.dma_start(th1[:], th1_d.ap())
            nc.sync.dma_start(m1i[:], m1i_d.ap())
            nc.sync.dma_start(cst2[:], cst2_d.ap())
            nc.sync.dma_start(nid[:], nid_d.ap())

            m1 = []
            s1_prev = []
            for m, (h0, h1) in enumerate(MCH):
                mw = h1 - h0
                mt = state.tile([128, BS], f32, tag=f"m1_{m}")
                nc.vector.memset(mt[:mw], 0.0)
                nc.vector.tensor_scalar(
                    mt[:mw], mt[:mw], m1i[:mw, m : m + 1], None, Alu.add
                )
                m1.append(mt)
                sp0 = s1pool.tile([128, BS], bf16, tag=f"s1_{m}")
                nc.gpsimd.memset(sp0[:mw], 0.0)
                s1_prev.append(sp0)

            m2 = state.tile([O, BS], f32, tag="m2")
            nc.vector.memset(m2[:], 0.0)
            nc.vector.tensor_scalar(m2[:], m2[:], cst2[:, 1:2], None, Alu.add)
            s2_prev = s2pool.tile([O, BS], bf16, tag="s2")
            nc.vector.memset(s2_prev[:], 0.0)

            for t in range(T):
                # ---- load this step's operands ----
                wh = wpool.tile([KD, NKD * H], bf16, tag="w1h")
                wl = wpool.tile([KD, NKD * H], bf16, tag="w1l")
                sp = wpool.tile([KD, NKD * BS], bf16, tag="sp")
                nc.sync.dma_start(wh[:], w1h_d.ap()[t])
                nc.gpsimd.dma_start(wl[:], w1l_d.ap()[t])
                nc.sync.dma_start(sp[:], sp_d.ap()[t])
                w2ht = wpool.tile([128, 40], bf16, tag="w2h")
                w2lt = wpool.tile([128, 40], bf16, tag="w2l")
                nc.gpsimd.dma_start(w2ht[:], w2h_d.ap()[t])
                nc.gpsimd.dma_start(w2lt[:], w2l_d.ap()[t])

                s1_new = []
                for m, (h0, h1) in enumerate(MCH):
                    mw = h1 - h0
                    ps = ps1.tile([128, BS], f32, tag="ps1")
                    # reset term folded into PE: psum -= spk_prev (via -I)
                    nc.tensor.matmul(
                        ps[:mw], nid[:mw, :mw], s1_prev[m][:mw, :],
                        start=True, stop=False,
                    )
                    for k in range(NKD):
                        for wsrc, last in ((wh, False), (wl, k == NKD - 1)):
                            nc.tensor.matmul(
                                ps[:mw],
                                wsrc[:, k * H + h0 : k * H + h1],
                                sp[:, k * BS : (k + 1) * BS],
                                start=False,
                                stop=last,
                            )
                    # m1' = 0.9*m1 + (cur - spk_prev) ; spk = m1' > th
                    u = upool.tile([128, BS], f32, tag=f"u_{m}")
                    nc.scalar.mul(u[:mw], m1[m][:mw], BETA)
                    nc.vector.tensor_tensor(m1[m][:mw], u[:mw], ps[:mw], op=Alu.add)
                    ns = s1pool.tile([128, BS], bf16, tag=f"s1_{m}")
                    nc.vector.tensor_scalar(
                        ns[:mw], m1[m][:mw], th1[:mw, m : m + 1], None, Alu.is_gt
                    )
                    s1_new.append(ns)

                # ---- layer 2: sequential accumulation, M=10, N=512 ----
                p2 = ps2.tile([O, BS], f32, tag="ps2")
                nc.tensor.matmul(
                    p2[:], nid[:O, :O], s2_prev[:], start=True, stop=False
                )
                for j in range(4):
                    kj = MCH[j][1] - MCH[j][0]
                    for wsrc, last in ((w2ht, False), (w2lt, j == 3)):
                        nc.tensor.matmul(
                            p2[:],
                            wsrc[:kj, j * 10 : (j + 1) * 10],
                            s1_new[j][:kj, :],
                            start=False,
                            stop=last,
                        )
                u2 = upool.tile([O, BS], f32, tag="u2")
                nc.scalar.mul(u2[:], m2[:], BETA)
                nc.vector.tensor_tensor(m2[:], u2[:], p2[:], op=Alu.add)
                s2 = s2pool.tile([O, BS], bf16, tag="s2")
                nc.vector.tensor_scalar(
                    s2[:], m2[:], cst2[:, 0:1], None, Alu.is_gt
                )
                mo = s2pool.tile([O, BS], f32, tag="m2o")
                nc.vector.tensor_scalar(
                    mo[:], m2[:], cst2[:, 2:3], None, Alu.add
                )
                nc.gpsimd.dma_start(spk2_d.ap()[:, t * BS : (t + 1) * BS], s2[:])
                nc.gpsimd.dma_start(mem2_d.ap()[:, t * BS : (t + 1) * BS], mo[:])
                s1_prev = s1_new
                s2_prev = s2

    nc.compile()
    return nc


_RNG_SCRIPT = r"""
import numpy as np, sys
import jax, jax.numpy as jnp
T,B,D,H,O = 25,4096,784,500,10
x = np.load(sys.argv[1])["x"]
key = jax.random.key(42)
k_sp, k_n1, k_n2 = jax.random.split(key, 3)
u = jax.random.uniform(k_sp, (T,B,D), dtype=jnp.float32)
spikes = np.asarray(u) < x[None]
n1 = np.asarray(jax.random.normal(k_n1, (T,D,H), dtype=jnp.float32) * 0.02)
n2 = np.asarray(jax.random.normal(k_n2, (T,H,O), dtype=jnp.float32) * 0.02)
np.savez(sys.argv[2], spikes=spikes, n1=n1, n2=n2)
"""


def _host_rng(x):
    """jax threefry must run in a pure-CPU process: under the axon plugin the
    RNG ops get routed to the neuron backend and yield a different stream."""
    import subprocess, tempfile, os

    with tempfile.TemporaryDirectory() as td:
        xp, op = os.path.join(td, "x.npz"), os.path.join(td, "rng.npz")
        np.savez(xp, x=x)
        env = dict(os.environ, JAX_PLATFORMS="cpu")
        subprocess.run(
            [sys.executable, "-c", _RNG_SCRIPT, xp, op],
            env=env, check=True, capture_output=True,
        )
        d = np.load(op)
        return d["spikes"], d["n1"], d["n2"]


def _prep_inputs(x, W1, b1, W2, b2):
    """Reproduce reference RNG on CPU and build per-core device inputs."""
    spikes, n1, n2 = _host_rng(np.asarray(x, np.float32))

    Wn1 = W1[None] + n1  # [T,D,H] f32
    Wn2 = W2[None] + n2  # [T,H,O] f32
    w1h = Wn1.astype(BF16)
    w1l = (Wn1 - w1h.astype(np.float32)).astype(BF16)
    # device layout [T, KD, NKD*H]: row p holds chunks k at free k*H+h
    w1h = np.ascontiguousarray(
        w1h.reshape(T, NKD, KD, H).transpose(0, 2, 1, 3).reshape(T, KD, NKD * H)
    )
    w1l = np.ascontiguousarray(
        w1l.reshape(T, NKD, KD, H).transpose(0, 2, 1, 3).reshape(T, KD, NKD * H)
    )

    w2h_f = Wn2.astype(BF16)
    w2l_f = (Wn2 - w2h_f.astype(np.float32)).astype(BF16)
    w2h = np.zeros((T, 128, 40), BF16)
    w2l = np.zeros((T, 128, 40), BF16)
    for j, (h0, h1) in enumerate(MCH):
        kj = h1 - h0
        w2h[:, :kj, j * 10 : (j + 1) * 10] = w2h_f[:, h0:h1, :]
        w2l[:, :kj, j * 10 : (j + 1) * 10] = w2l_f[:, h0:h1, :]

    th1 = np.zeros((128, 4), np.float32)
    m1i = np.zeros((128, 4), np.float32)
    for m, (h0, h1) in enumerate(MCH):
        mw = h1 - h0
        th1[:mw, m] = THRESHOLD - 10.0 * b1[h0:h1]
        m1i[:mw, m] = -10.0 * b1[h0:h1]
    cst2 = np.stack(
        [THRESHOLD - 10.0 * b2, -10.0 * b2, 10.0 * b2], axis=1
    ).astype(np.float32)

    nid = np.zeros((128, 128), BF16)
    np.fill_diagonal(nid, BF16(-1.0))
    shared = {
        "w1h": w1h, "w1l": w1l, "w2h": w2h, "w2l": w2l,
        "th1": th1, "m1i": m1i, "cst2": cst2, "nid": nid,
    }
    in_maps = []
    for c in range(NCORES):
        sl = spikes[:, c * BS : (c + 1) * BS, :]  # [T,BS,D]
        spT = sl.transpose(0, 2, 1).astype(BF16)  # [T,D,BS]
        spT = np.ascontiguousarray(
            spT.reshape(T, NKD, KD, BS).transpose(0, 2, 1, 3).reshape(T, KD, NKD * BS)
        )
        in_maps.append({"sp": spT, **shared})
    return in_maps


def _install_trace_shim():
    """This image's antenv lacks axon_hooks; recreate the NTFF hook from
    trn_boot's ctypes wrapper so run_bass_kernel_spmd(trace=True) works."""
    import types
    from concourse import bass_utils

    if "antenv.axon_hooks" not in sys.modules:
        from trn_agent_boot.trn_boot import _ntff_profile_via_ctypes

        hook = _ntff_profile_via_ctypes("/opt/axon/libaxon_pjrt.so")
        m = types.ModuleType("antenv.axon_hooks")
        m.get_axon_ntff_profile_hook = lambda: hook
        m.set_axon_ntff_profile_hook = lambda h: None
        sys.modules["antenv.axon_hooks"] = m
    bass_utils.upload_artifacts = lambda tmpdir: tmpdir


def _run(inputs, trace=False, trace_kwargs=None):
    from concourse import bass_utils

    if trace:
        _install_trace_shim()
    if "nc" not in _CACHE:
        _CACHE["nc"] = _build()
    nc = _CACHE["nc"]
    in_maps = _prep_inputs(
        inputs["x"], inputs["W1"], inputs["b1"], inputs["W2"], inputs["b2"]
    )
    res = bass_utils.run_bass_kernel_spmd(
        nc,
        in_maps,
        core_ids=list(range(NCORES)),
        trace=trace,
        **(trace_kwargs or {}),
    )
    spk2 = np.empty((T, B, O), np.float32)
    mem2 = np.empty((T, B, O), np.float32)
    for c in range(NCORES):
        s = res.results[c]["spk2o"].astype(np.float32).reshape(O, T, BS).transpose(1, 2, 0)
        m = res.results[c]["mem2o"].reshape(O, T, BS).transpose(1, 2, 0)
        spk2[:, c * BS : (c + 1) * BS, :] = s
        mem2[:, c * BS : (c + 1) * BS, :] = m
    return (spk2, mem2), res


def kernel(x, W1, b1, W2, b2):
    (spk2, mem2), _ = _run(
        {"x": x, "W1": W1, "b1": b1, "W2": W2, "b2": b2}, trace=False
    )
    return spk2, mem2


# revision 19
# speedup vs baseline: 2.5479x; 1.0202x over previous
"""Hardware-aware SNN (LIF, 2-layer, per-step weight noise) on 8 TRN2 cores.

Strategy
--------
Data-parallel over batch (4096 -> 8 x 512). Host reproduces the reference's
jax.random streams bit-exactly on CPU (threefry is backend-invariant),
precomputes per-step noisy weights Wn1[t] = W1 + e1[t] (f32) and splits them
into bf16 hi + bf16 lo so the PE runs full-rate bf16 matmuls with ~2^-18
effective weight precision. Spikes are 0/1 -> exact in bf16.

Device kernel (per core), per step t:
  layer1: psum[m] = sum_k  Wn1hi/lo[t,k,m].T @ spikesT[t,k]   (H-part x B-free)
  recurrence (shifted membrane mt = mem - 10*b so bias vanishes;
              reset term r_{t+1} == spk_t exactly):
     u = 0.9*mt (ACT);  u += psum (DVE);  mt = u - spk_prev (DVE)
     spk = (mt > theta) per-partition theta = 1 - 10*b   (GPSIMD, bf16 out)
  layer2: 4 col-strip-packed matmuls (tile_position) x hi/lo into one psum,
          strip-sum + same recurrence on [10,512]; un-shift mem2 for output.
Outputs are written [10, T*B] and transposed on host.
"""
import sys

sys.path.insert(0, "/opt/trn_rl_repo")

import numpy as np
import ml_dtypes

T = 25
B, D, H, O = 4096, 784, 500, 10
BETA = 0.9
THRESHOLD = 1.0
NOISE_STD = 0.02
NCORES = 8
BS = B // NCORES  # 512 batch per core
KD = 112          # D contraction chunk (784 = 7*112)
NKD = 7
MCH = [(0, 128), (128, 256), (256, 384), (384, 500)]  # H chunks
BF16 = ml_dtypes.bfloat16

_CACHE = {}


def _build():
    import concourse.bass as bass
    import concourse.bacc as bacc
    import concourse.mybir as mybir
    import concourse.tile as tile

    f32 = mybir.dt.float32
    bf16 = mybir.dt.bfloat16
    Alu = mybir.AluOpType

    nc = bacc.Bacc("TRN2", target_bir_lowering=False, debug=False)

    # w1 layout: 13 blocks of H — blocks 0-5 hi chunks (128 d-rows each),
    # 6-11 lo chunks, 12 = tail (rows 0-15 hi[768:784], 16-31 lo[768:784]).
    # sp layout: 7 blocks of BS — blocks 0-5 d-chunks, 6 = dup'd tail rows.
    sp_d = nc.dram_tensor("sp", [T, 128, 7 * BS], bf16, kind="ExternalInput")
    w1_d = nc.dram_tensor("w1", [T, 128, 13 * H], bf16, kind="ExternalInput")
    w2h_d = nc.dram_tensor("w2h", [T, 128, 40], bf16, kind="ExternalInput")
    w2l_d = nc.dram_tensor("w2l", [T, 128, 40], bf16, kind="ExternalInput")
    th1_d = nc.dram_tensor("th1", [128, 4], f32, kind="ExternalInput")
    m1i_d = nc.dram_tensor("m1i", [128, 4], f32, kind="ExternalInput")
    cst2_d = nc.dram_tensor("cst2", [O, 3], f32, kind="ExternalInput")
    nid_d = nc.dram_tensor("nid", [128, 128], bf16, kind="ExternalInput")
    spk2_d = nc.dram_tensor("spk2o", [O, T * BS], bf16, kind="ExternalOutput")
    mem2_d = nc.dram_tensor("mem2o", [O, T * BS], f32, kind="ExternalOutput")

    with tile.TileContext(nc) as tc:
        with (
            tc.tile_pool(name="wpool", bufs=4) as wpool,
            tc.tile_pool(name="state", bufs=1) as state,
            tc.tile_pool(name="upool", bufs=2) as upool,
            tc.tile_pool(name="s1pool", bufs=2) as s1pool,
            tc.tile_pool(name="s2pool", bufs=2) as s2pool,
            tc.tile_pool(name="ps1", bufs=6, space="PSUM") as ps1,
            tc.tile_pool(name="ps2", bufs=2, space="PSUM") as ps2,
        ):
            # ---- constants / state init ----
            th1 = state.tile([128, 4], f32, tag="th1")
            m1i = state.tile([128, 4], f32, tag="m1i")
            cst2 = state.tile([O, 3], f32, tag="cst2")  # cols: th2, m2i, 10*b2
            nid = state.tile([128, 128], bf16, tag="nid")
            nc.sync.dma_start(th1[:], th1_d.ap())
            nc.sync.dma_start(m1i[:], m1i_d.ap())
            nc.sync.dma_start(cst2[:], cst2_d.ap())
            nc.sync.dma_start(nid[:], nid_d.ap())

            m1 = []
            s1_prev = []
            for m, (h0, h1) in enumerate(MCH):
                mw = h1 - h0
                mt = state.tile([128, BS], f32, tag=f"m1_{m}")
                nc.vector.memset(mt[:mw], 0.0)
                nc.vector.tensor_scalar(
                    mt[:mw], mt[:mw], m1i[:mw, m : m + 1], None, Alu.add
                )
                m1.append(mt)
                sp0 = s1pool.tile([128, BS], bf16, tag=f"s1_{m}")
                nc.gpsimd.memset(sp0[:mw], 0.0)
                s1_prev.append(sp0)

            m2 = state.tile([O, BS], f32, tag="m2")
            nc.vector.memset(m2[:], 0.0)
            nc.vector.tensor_scalar(m2[:], m2[:], cst2[:, 1:2], None, Alu.add)
            s2_prev = s2pool.tile([O, BS], bf16, tag="s2")
            nc.vector.memset(s2_prev[:], 0.0)

            for t in range(T):
                # ---- load this step's operands ----
                w1t = wpool.tile([128, 13 * H], bf16, tag="w1")
                sp = wpool.tile([128, 7 * BS], bf16, tag="sp")
                nc.sync.dma_start(w1t[:], w1_d.ap()[t])
                nc.sync.dma_start(sp[:], sp_d.ap()[t])
                w2ht = wpool.tile([128, 40], bf16, tag="w2h")
                w2lt = wpool.tile([128, 40], bf16, tag="w2l")
                nc.gpsimd.dma_start(w2ht[:], w2h_d.ap()[t])
                nc.gpsimd.dma_start(w2lt[:], w2l_d.ap()[t])

                s1_new = []
                for m, (h0, h1) in enumerate(MCH):
                    mw = h1 - h0
                    ps = ps1.tile([128, BS], f32, tag="ps1")
                    # reset term folded into PE: psum -= spk_prev (via -I)
                    nc.tensor.matmul(
                        ps[:mw], nid[:mw, :mw], s1_prev[m][:mw, :],
                        start=True, stop=False,
                    )
                    for c in range(12):  # 6 hi + 6 lo full 128-row chunks
                        nc.tensor.matmul(
                            ps[:mw],
                            w1t[:, c * H + h0 : c * H + h1],
                            sp[:, (c % 6) * BS : (c % 6 + 1) * BS],
                            start=False,
                            stop=False,
                        )
                    # combined 32-row tail: hi[768:784] + lo[768:784]
                    nc.tensor.matmul(
                        ps[:mw],
                        w1t[:32, 12 * H + h0 : 12 * H + h1],
                        sp[:32, 6 * BS : 7 * BS],
                        start=False,
                        stop=True,
                    )
                    # m1' = 0.9*m1 + (cur - spk_prev) ; spk = m1' > th
                    u = upool.tile([128, BS], f32, tag=f"u_{m}")
                    nc.scalar.mul(u[:mw], m1[m][:mw], BETA)
                    nc.vector.tensor_tensor(m1[m][:mw], u[:mw], ps[:mw], op=Alu.add)
                    ns = s1pool.tile([128, BS], bf16, tag=f"s1_{m}")
                    nc.vector.tensor_scalar(
                        ns[:mw], m1[m][:mw], th1[:mw, m : m + 1], None, Alu.is_gt
                    )
                    s1_new.append(ns)

                # ---- layer 2: 4-way col-strip packed, reset in strip 1 ----
                p2 = ps2.tile([128, BS], f32, tag="ps2")
                for j in range(4):
                    kj = MCH[j][1] - MCH[j][0]
                    nc.tensor.matmul(
                        p2[32 * j : 32 * j + O, :],
                        w2ht[:kj, j * 10 : (j + 1) * 10],
                        s1_new[j][:kj, :],
                        start=True, stop=False,
                        tile_position=(0, 32 * j),
                    )
                    nc.tensor.matmul(
                        p2[32 * j : 32 * j + O, :],
                        w2lt[:kj, j * 10 : (j + 1) * 10],
                        s1_new[j][:kj, :],
                        start=False, stop=(j != 1),
                        tile_position=(0, 32 * j),
                    )
                nc.tensor.matmul(
                    p2[32 : 32 + O, :], nid[:O, :O], s2_prev[:],
                    start=False, stop=True, tile_position=(0, 32),
                )
                a = upool.tile([O, BS], f32, tag="l2a")
                nc.scalar.copy(a[:], p2[0:O, :])
                nc.vector.tensor_tensor(a[:], a[:], p2[32 : 32 + O, :], op=Alu.add)
                nc.vector.tensor_tensor(a[:], a[:], p2[64 : 64 + O, :], op=Alu.add)
                nc.vector.tensor_tensor(a[:], a[:], p2[96 : 96 + O, :], op=Alu.add)
                u2 = upool.tile([O, BS], f32, tag="u2")
                nc.scalar.mul(u2[:], m2[:], BETA)
                nc.vector.tensor_tensor(m2[:], u2[:], a[:], op=Alu.add)
                s2 = s2pool.tile([O, BS], bf16, tag="s2")
                nc.vector.tensor_scalar(
                    s2[:], m2[:], cst2[:, 0:1], None, Alu.is_gt
                )
                mo = s2pool.tile([O, BS], f32, tag="m2o")
                nc.vector.tensor_scalar(
                    mo[:], m2[:], cst2[:, 2:3], None, Alu.add
                )
                nc.gpsimd.dma_start(spk2_d.ap()[:, t * BS : (t + 1) * BS], s2[:])
                nc.gpsimd.dma_start(mem2_d.ap()[:, t * BS : (t + 1) * BS], mo[:])
                s1_prev = s1_new
                s2_prev = s2

    nc.compile()
    return nc


_RNG_SCRIPT = r"""
import numpy as np, sys
import jax, jax.numpy as jnp
T,B,D,H,O = 25,4096,784,500,10
x = np.load(sys.argv[1])["x"]
key = jax.random.key(42)
k_sp, k_n1, k_n2 = jax.random.split(key, 3)
u = jax.random.uniform(k_sp, (T,B,D), dtype=jnp.float32)
spikes = np.asarray(u) < x[None]
n1 = np.asarray(jax.random.normal(k_n1, (T,D,H), dtype=jnp.float32) * 0.02)
n2 = np.asarray(jax.random.normal(k_n2, (T,H,O), dtype=jnp.float32) * 0.02)
np.savez(sys.argv[2], spikes=spikes, n1=n1, n2=n2)
"""


def _host_rng(x):
    """jax threefry must run in a pure-CPU process: under the axon plugin the
    RNG ops get routed to the neuron backend and yield a different stream."""
    import subprocess, tempfile, os

    with tempfile.TemporaryDirectory() as td:
        xp, op = os.path.join(td, "x.npz"), os.path.join(td, "rng.npz")
        np.savez(xp, x=x)
        env = dict(os.environ, JAX_PLATFORMS="cpu")
        subprocess.run(
            [sys.executable, "-c", _RNG_SCRIPT, xp, op],
            env=env, check=True, capture_output=True,
        )
        d = np.load(op)
        return d["spikes"], d["n1"], d["n2"]


def _prep_inputs(x, W1, b1, W2, b2):
    """Reproduce reference RNG on CPU and build per-core device inputs."""
    spikes, n1, n2 = _host_rng(np.asarray(x, np.float32))

    Wn1 = W1[None] + n1  # [T,D,H] f32
    Wn2 = W2[None] + n2  # [T,H,O] f32
    w1h = Wn1.astype(BF16)
    w1l = (Wn1 - w1h.astype(np.float32)).astype(BF16)
    # blocks 0-5: hi 128-row chunks; 6-11: lo; 12: 32-row combined tail
    w1all = np.zeros((T, 128, 13 * H), BF16)
    w1all[:, :, : 6 * H] = (
        w1h[:, :768].reshape(T, 6, 128, H).transpose(0, 2, 1, 3).reshape(T, 128, 6 * H)
    )
    w1all[:, :, 6 * H : 12 * H] = (
        w1l[:, :768].reshape(T, 6, 128, H).transpose(0, 2, 1, 3).reshape(T, 128, 6 * H)
    )
    w1all[:, :16, 12 * H :] = w1h[:, 768:784]
    w1all[:, 16:32, 12 * H :] = w1l[:, 768:784]

    w2h_f = Wn2.astype(BF16)
    w2l_f = (Wn2 - w2h_f.astype(np.float32)).astype(BF16)
    w2h = np.zeros((T, 128, 40), BF16)
    w2l = np.zeros((T, 128, 40), BF16)
    for j, (h0, h1) in enumerate(MCH):
        kj = h1 - h0
        w2h[:, :kj, j * 10 : (j + 1) * 10] = w2h_f[:, h0:h1, :]
        w2l[:, :kj, j * 10 : (j + 1) * 10] = w2l_f[:, h0:h1, :]

    th1 = np.zeros((128, 4), np.float32)
    m1i = np.zeros((128, 4), np.float32)
    for m, (h0, h1) in enumerate(MCH):
        mw = h1 - h0
        th1[:mw, m] = THRESHOLD - 10.0 * b1[h0:h1]
        m1i[:mw, m] = -10.0 * b1[h0:h1]
    cst2 = np.stack(
        [THRESHOLD - 10.0 * b2, -10.0 * b2, 10.0 * b2], axis=1
    ).astype(np.float32)

    nid = np.zeros((128, 128), BF16)
    np.fill_diagonal(nid, BF16(-1.0))
    shared = {
        "w1": w1all, "w2h": w2h, "w2l": w2l,
        "th1": th1, "m1i": m1i, "cst2": cst2, "nid": nid,
    }
    in_maps = []
    for c in range(NCORES):
        sl = spikes[:, c * BS : (c + 1) * BS, :]  # [T,BS,D]
        spT = sl.transpose(0, 2, 1).astype(BF16)  # [T,D,BS]
        spall = np.zeros((T, 128, 7 * BS), BF16)
        spall[:, :, : 6 * BS] = (
            spT[:, :768].reshape(T, 6, 128, BS).transpose(0, 2, 1, 3)
            .reshape(T, 128, 6 * BS)
        )
        spall[:, :16, 6 * BS :] = spT[:, 768:784]
        spall[:, 16:32, 6 * BS :] = spT[:, 768:784]
        in_maps.append({"sp": spall, **shared})
    return in_maps


def _install_trace_shim():
    """This image's antenv lacks axon_hooks; recreate the NTFF hook from
    trn_boot's ctypes wrapper so run_bass_kernel_spmd(trace=True) works."""
    import types
    from concourse import bass_utils

    if "antenv.axon_hooks" not in sys.modules:
        from trn_agent_boot.trn_boot import _ntff_profile_via_ctypes

        hook = _ntff_profile_via_ctypes("/opt/axon/libaxon_pjrt.so")
        m = types.ModuleType("antenv.axon_hooks")
        m.get_axon_ntff_profile_hook = lambda: hook
        m.set_axon_ntff_profile_hook = lambda h: None
        sys.modules["antenv.axon_hooks"] = m
    bass_utils.upload_artifacts = lambda tmpdir: tmpdir


def _run(inputs, trace=False, trace_kwargs=None):
    from concourse import bass_utils

    if trace:
        _install_trace_shim()
    if "nc" not in _CACHE:
        _CACHE["nc"] = _build()
    nc = _CACHE["nc"]
    in_maps = _prep_inputs(
        inputs["x"], inputs["W1"], inputs["b1"], inputs["W2"], inputs["b2"]
    )
    res = bass_utils.run_bass_kernel_spmd(
        nc,
        in_maps,
        core_ids=list(range(NCORES)),
        trace=trace,
        **(trace_kwargs or {}),
    )
    spk2 = np.empty((T, B, O), np.float32)
    mem2 = np.empty((T, B, O), np.float32)
    for c in range(NCORES):
        s = res.results[c]["spk2o"].astype(np.float32).reshape(O, T, BS).transpose(1, 2, 0)
        m = res.results[c]["mem2o"].reshape(O, T, BS).transpose(1, 2, 0)
        spk2[:, c * BS : (c + 1) * BS, :] = s
        mem2[:, c * BS : (c + 1) * BS, :] = m
    return (spk2, mem2), res


def kernel(x, W1, b1, W2, b2):
    (spk2, mem2), _ = _run(
        {"x": x, "W1": W1, "b1": b1, "W2": W2, "b2": b2}, trace=False
    )
    return spk2, mem2
